# revision 8
# baseline (speedup 1.0000x reference)
"""Trainium2 Bass kernel for GQA causal sliding-window self-attention.

Problem: B=2, T=2048, C=1024, 16 heads (hd=64), 4 KV groups, window=256.

Sharding: data-parallel over (batch, T-chunk): 8 cores = 2 batches x 4
chunks of 512 query tokens; each core gets a 768-token extended x slice
(512 queries + 256 halo) and computes its output rows locally.

Design: all-bf16 operands (halves DMA, 1 cyc/col matmuls at any free
size), diagonal 128-query attention blocks (384-key span = 1.5x
overcompute instead of 2x), 2-head-packed score matmuls (qT stored
[64, 2, TQ] per head-pair, kT [64, NG, TE], all matmul operands at base
partition 0), TRANSPOSED av (queries on partitions; ones column in v
gives softmax denominators) so the reciprocal is per-partition and
normalization is one stride-0-broadcast DVE multiply, then a transpose
back to head-major via a plain bf16 matmul against an identity matrix.
Band masks are multiplicative 0/1 bf16 constants (GPSIMD kt0 +
qb0-edge kt1; DVE kt2; the middle kt tile needs no mask), applied to
the exp'd scores. Output is bf16, upcast on host; host folds 1/8 into
Wq/bq and adds the exact linear bv/bo correction (bv_rep @ Wo + bo).

PSUM (8 banks): scores [128,3,4,128] x2 bufs (6), combined
y_t[128,4,65]+bf16 transpose region x1 (1), shared proj/out bank (1).
Early projections borrow the scores pool before attention starts.

Pipeline: per unit (group, 128-query block): scores -> exp (ScalarE)
-> masks -> av -> recip/norm (DVE) -> transpose -> yn copy, with
q/v-projection and out-projection chunks interleaved as PE filler.

Environment constraints found the hard way (this walrus/axon build):
 - max 1 sync wait per instruction (_split_multi_waits hoists extras
   onto same-engine NOPs)
 - bf16 matmuls with operands at base partition 64 crash the compile;
   all operands must sit at partition 0 (hence qT/kT layouts)
 - is_transpose=True matmuls return wrong data on hw (interp-only)
 - only one open PSUM accumulation group per 2KB bank
 - GPSIMD cannot touch PSUM; DVE/ScalarE cross-partition-offset ok
 - DMA cannot read PSUM
"""

import sys

sys.path.insert(0, "/opt/trn_rl_repo")

import contextlib

import numpy as np

import concourse.bass as bass
import concourse.tile as tile
from concourse import mybir
from concourse.bass_utils import run_bass_kernel_spmd
from concourse.vector_clock import ScopedClock

F32 = mybir.dt.float32
BF16 = mybir.dt.bfloat16
F8E4 = mybir.dt.float8e4
FP8_SCALE = 32.0  # weights *32 into fp8 normal range; q carries the 1/32

import os

USE_IS_TRANSPOSE = os.environ.get("KV2_ISTR", "") == "1"  # wrong results on hw
USE_BCAST = os.environ.get("KV2_NO_BCAST", "") != "1"
USE_SC_IDBIAS = os.environ.get("KV2_NO_IDBIAS", "") != "1"

B, T, C = 2, 2048, 1024
NH, NG, HD = 16, 4, 64
KV = NG * HD  # 256
WINDOW = 256
NCORES = 8
TQ = 512  # query tokens per core
TE = TQ + WINDOW  # 768 extended tokens per core
KC = C // 128  # 8 contraction tiles


class _ChunkedDrainTileContext(tile.TileContext):
    """Walrus in this container only accepts 1 sync wait on CTRL-class
    instructions; spread the tail drain waits over engine NOPs."""

    def _drain_and_barrier(self, tick_clock, wait_clock):
        gc = tick_clock.global_clock
        entries = []
        for scope, vc in ScopedClock({None: gc}).items():
            for proc in range(len(vc)):
                t = vc[proc]
                if t > 0:
                    entries.append((scope, proc, t))
        engines = [self.nc.sync, self.nc.vector, self.nc.scalar, self.nc.gpsimd]
        curs = [ScopedClock() for _ in engines]
        for i, (scope, proc, t) in enumerate(entries):
            eng = engines[i % len(engines)]
            nop = eng.nop(nofuse=True, hint="tail_wait")
            partial = ScopedClock()
            partial.require_at_least(scope, proc, t)
            wait_clock.add_sem_waits(nop.ins, partial, curs[i % len(engines)])
            curs[i % len(engines)].update_past(partial)
        self.nc.all_engine_barrier(sem_only=True)
        drain_inst = self.nc.sync.drain()
        cur = ScopedClock()
        for c in curs:
            cur.update_past(c)
        wait_clock.add_sem_waits(drain_inst.ins, ScopedClock({None: gc}), cur)
        assert self.sems is not None
        popped = self.nc._tile_sem_poison_stack.pop()
        assert popped is self._sem_poison
        self.nc.clear_and_free_semaphores(list(self.sems.allocated().values()))


def _split_multi_waits(nc, max_waits=1):
    """Hoist excess sync waits onto same-engine NOPs (walrus quirk)."""
    fn = nc.m.functions[0]
    for blk in fn.blocks:
        insts = blk.instructions
        new = []
        changed = False
        for inst in insts:
            si = inst.sync_info
            waits = list(si.on_wait) if si is not None and si.on_wait else []
            if len(waits) > max_waits:
                changed = True
                for w in waits[:-max_waits]:
                    nop = mybir.InstNoOp(
                        name=nc.get_next_instruction_name(),
                        ins=[],
                        outs=[],
                        engine=inst.engine,
                        sync_info=mybir.SyncInfo(on_wait=[w], on_update=[]),
                        bass_nofuse=True,
                    )
                    nc.register_instruction(nop, overwrite=True)
                    new.append(nop)
                si.on_wait = waits[-max_waits:]
                inst.sync_info = si
            new.append(inst)
        if changed:
            blk.instructions = new


def _build_program():
    nc = bass.Bass("TRN2", target_bir_lowering=False, debug=False, num_devices=NCORES)

    xt = nc.dram_tensor("xt", [128, KC, TE], BF16, kind="ExternalInput")
    wq = nc.dram_tensor("wq", [KC, 128, KC, 128], BF16, kind="ExternalInput")
    wk = nc.dram_tensor("wk", [128, KC, KV], BF16, kind="ExternalInput")
    wv = nc.dram_tensor("wv", [128, KC, KV], BF16, kind="ExternalInput")
    wo = nc.dram_tensor("wo", [128, KC, C], BF16, kind="ExternalInput")
    bq = nc.dram_tensor("bq", [128, KC], F32, kind="ExternalInput")
    bk = nc.dram_tensor("bk", [128, 2], F32, kind="ExternalInput")
    ident = nc.dram_tensor("ident", [128, 128], BF16, kind="ExternalInput")
    # mask slots: 0 = kt0@qb0, 1 = kt1@qb0, 2 = kt0@qb>0 (T0), 3 = kt2 (T2)
    maskp = nc.dram_tensor("maskp", [128, 4, NG, 128], BF16, kind="ExternalInput")
    out = nc.dram_tensor("out", [TQ, C], BF16, kind="ExternalOutput")

    with _ChunkedDrainTileContext(nc) as tc:
        with contextlib.ExitStack() as ctx:
            wsb = ctx.enter_context(tc.tile_pool(name="wsb", bufs=1))
            xsb = ctx.enter_context(tc.tile_pool(name="xsb", bufs=1))
            csb = ctx.enter_context(tc.tile_pool(name="csb", bufs=1))
            qkv = ctx.enter_context(tc.tile_pool(name="qkv", bufs=1))
            ynp = ctx.enter_context(tc.tile_pool(name="ynp", bufs=1))
            expp = ctx.enter_context(tc.tile_pool(name="expp", bufs=8))
            rcpp = ctx.enter_context(tc.tile_pool(name="rcpp", bufs=4))
            ytnp = ctx.enter_context(tc.tile_pool(name="ytnp", bufs=6))
            obp = ctx.enter_context(tc.tile_pool(name="obp", bufs=3))
            pj = ctx.enter_context(tc.tile_pool(name="pj", bufs=1, space="PSUM"))
            # av-output y_t [128,4,65] gets its own bank
            ytp_pool = ctx.enter_context(tc.tile_pool(name="ytp", bufs=1, space="PSUM"))
            scp_pool = ctx.enter_context(tc.tile_pool(name="scp", bufs=2, space="PSUM"))

            EARLY = [True]

            def proj_psum():
                # before the attention pipeline starts, projections borrow
                # the scores pool's big tiles (double-buffered); once units
                # are flowing they use the dedicated pj bank so they don't
                # steal the scores rotation.
                if EARLY[0]:
                    t = scp_pool.tile([128, 3, NG, 128], F32, name="scp", tag="sc")
                    return t[:, 0, :, :].rearrange("p g q -> p (g q)")
                return pj.tile([128, 512], F32, name="pp", tag="pj")

            # ---- loads, ordered by consumption deadline ----
            wk_sb = wsb.tile([128, KC, KV], BF16, name="wk_sb", tag="wk")
            nc.sync.dma_start(out=wk_sb[:, 0:4, 0:128], in_=wk[:, 0:4, 0:128])
            xt_sb = xsb.tile([128, KC, TE], BF16, name="xt_sb", tag="xt")
            nc.sync.dma_start(out=xt_sb[:, 0:2, 0:384], in_=xt[:, 0:2, 0:384])
            nc.sync.dma_start(out=xt_sb[:, 2:4, 0:384], in_=xt[:, 2:4, 0:384])
            nc.sync.dma_start(out=wk_sb[:, 4:8, 0:128], in_=wk[:, 4:8, 0:128])
            nc.sync.dma_start(out=xt_sb[:, 4:6, 0:384], in_=xt[:, 4:6, 0:384])
            nc.sync.dma_start(out=xt_sb[:, 6:8, 0:384], in_=xt[:, 6:8, 0:384])
            nc.sync.dma_start(out=wk_sb[:, :, 128:256], in_=wk[:, :, 128:256])
            bk_sb = csb.tile([128, 2], F32)
            nc.sync.dma_start(out=bk_sb[:], in_=bk[:])
            wv_sb = wsb.tile([128, KC, KV], BF16, name="wv_sb", tag="wv")
            nc.sync.dma_start(out=wv_sb[:], in_=wv[:])
            nc.sync.dma_start(out=xt_sb[:, :, 384:TE], in_=xt[:, :, 384:TE])
            bq_sb = csb.tile([128, KC], F32)
            nc.sync.dma_start(out=bq_sb[:], in_=bq[:])

            wq_sb = [None] * KC

            def _load_wq(m):
                t = wsb.tile([128, KC, 128], BF16, name=f"wq{m}", tag=f"wq{m}")
                nc.sync.dma_start(out=t[:], in_=wq[m, :, :, :])
                wq_sb[m] = t

            for m in range(2):
                _load_wq(m)
            id_sb = csb.tile([128, 128], BF16)
            nc.sync.dma_start(out=id_sb[:], in_=ident[:])
            mask_sb = csb.tile([128, 4, NG, 128], BF16)
            nc.sync.dma_start(out=mask_sb[:], in_=maskp[:])
            for m in range(2, KC):
                _load_wq(m)
            wo_sb = wsb.tile([128, KC, C], BF16, name="wo_sb", tag="wo")
            nc.sync.dma_start(out=wo_sb[:, :, 0:512], in_=wo[:, :, 0:512])
            nc.sync.dma_start(out=wo_sb[:, :, 512:1024], in_=wo[:, :, 512:1024])

            # ---- kT projection: kT4 [64, NG, TE] bf16 (base-partition 0;
            # bf16 matmuls with operands at base partition 64 crash walrus)
            kT4 = qkv.tile([64, NG, TE], BF16, name="kT4", tag="kT4")

            def k_proj(s2, mt):
                kp = proj_psum()
                for kc in range(KC):
                    nc.tensor.matmul(
                        kp[:, 0:384],
                        wk_sb[:, kc, mt * 128 : (mt + 1) * 128],
                        xt_sb[:, kc, s2 * 384 : (s2 + 1) * 384],
                        start=(kc == 0),
                        stop=(kc == KC - 1),
                    )
                for gh in range(2):
                    g = 2 * mt + gh
                    if gh == 0:
                        nc.vector.tensor_scalar_add(
                            kT4[0:64, g, s2 * 384 : (s2 + 1) * 384],
                            kp[0:64, 0:384],
                            bk_sb[0:64, mt : mt + 1],
                        )
                    else:
                        nc.scalar.activation(
                            kT4[0:64, g, s2 * 384 : (s2 + 1) * 384],
                            kp[64:128, 0:384],
                            mybir.ActivationFunctionType.Identity,
                            bias=bk_sb[64:128, mt : mt + 1],
                        )

            # ---- v projection: token-major [128, NG, 65] with ones column ----
            v_sb = []
            for vt in range(6):
                t = qkv.tile([128, NG, HD + 1], BF16, name=f"v{vt}", tag=f"v{vt}")
                nc.vector.memset(t[:, :, HD : HD + 1], 1.0)
                v_sb.append(t)

            def v_proj(vt):
                vp = proj_psum()
                for kc in range(KC):
                    nc.tensor.matmul(
                        vp[:, 0:KV],
                        xt_sb[:, kc, vt * 128 : (vt + 1) * 128],
                        wv_sb[:, kc, :],
                        start=(kc == 0),
                        stop=(kc == KC - 1),
                    )
                nc.scalar.copy(
                    v_sb[vt][:, :, 0:HD],
                    vp[:, 0:KV].rearrange("p (g d) -> p g d", g=NG),
                )

            yn = ynp.tile([128, KC, TQ], BF16)
            qT_sb = [None] * KC

            def q_proj(m):
                qp = proj_psum()
                for kc in range(KC):
                    nc.tensor.matmul(
                        qp[:],
                        wq_sb[m][:, kc, :],
                        xt_sb[:, kc, WINDOW:TE],
                        start=(kc == 0),
                        stop=(kc == KC - 1),
                    )
                qT = qkv.tile([64, 2, TQ], BF16, name=f"qT{m}", tag=f"qT{m}")
                nc.vector.tensor_scalar_add(
                    qT[0:64, 0, :], qp[0:64, :], bq_sb[0:64, m : m + 1]
                )
                nc.scalar.activation(
                    qT[0:64, 1, :],
                    qp[64:128, :],
                    mybir.ActivationFunctionType.Identity,
                    bias=bq_sb[64:128, m : m + 1],
                )
                qT_sb[m] = qT

            def attn_scores(g, qb):
                """12 score matmuls + exp + masks; returns masked ex tile."""
                scp = scp_pool.tile([128, 3, NG, 128], F32, name="scp", tag="sc")
                for kt in range(3):
                    ke0 = qb * 128 + kt * 128
                    for mi in range(2):
                        m = 2 * g + mi
                        # 2 heads per matmul: rhs [64, 2, 128] (hi = 2*mi+hh)
                        nc.tensor.matmul(
                            scp[:, kt, 2 * mi : 2 * mi + 2, :],
                            kT4[0:64, g, ke0 : ke0 + 128],
                            qT_sb[m][0:64, :, qb * 128 : qb * 128 + 128],
                            start=True,
                            stop=True,
                        )
                ex = expp.tile([128, 3, NG, 128], BF16, name="ex", tag="ex")
                nc.scalar.activation(ex[:], scp[:], mybir.ActivationFunctionType.Exp)
                return ex

            def attn_masks(g, qb, ex):
                # band masks (multiplicative 0/1): kt0 always, kt1 only at qb0,
                # kt2 always. Middle tile fully valid for qb>0. Emitted AFTER
                # the previous unit's recip/norm so they don't head-of-line
                # block the DVE queue while waiting on exp.
                # edge cores: qb0 kt0/kt1 fully padded; qb1 kt0 also reaches
                # into the padding (slot 0 = T0 on std cores, zeros on edge)
                slot0 = 0 if qb <= 1 else 2
                nc.vector.tensor_tensor(
                    ex[:, 0, :, :], ex[:, 0, :, :], mask_sb[:, slot0, :, :],
                    mybir.AluOpType.mult,
                )
                if qb == 0:
                    nc.gpsimd.tensor_tensor(
                        ex[:, 1, :, :], ex[:, 1, :, :], mask_sb[:, 1, :, :],
                        mybir.AluOpType.mult,
                    )
                nc.vector.tensor_tensor(
                    ex[:, 2, :, :], ex[:, 2, :, :], mask_sb[:, 3, :, :],
                    mybir.AluOpType.mult,
                )

            def attn_av(g, qb, ex):
                """av (transposed) + recip + norm. kt order 1,2,0 so the
                unmasked middle tile starts immediately after exp and the
                slow gpsimd kt0 mask gets maximal slack."""
                comb = ytp_pool.tile([128, 388], F32, name="comb", tag="yt")
                y_t = comb[:, 0:260].rearrange("p (h d) -> p h d", h=NG)
                # hi outer: only one psum accumulation group open per bank
                for hi in range(4):
                    for kt in (1, 0, 2):
                        nc.tensor.matmul(
                            y_t[:, hi, :],
                            ex[:, kt, hi, :],
                            v_sb[qb + kt][:, g, :],
                            start=(kt == 1),
                            stop=(kt == 2),
                        )

                rcp = rcpp.tile([128, NG, 1], F32, name="rcp", tag="rcp")
                with nc.allow_low_precision(reason="softmax denom reciprocal"):
                    nc.vector.reciprocal(rcp[:], y_t[:, :, HD : HD + 1])
                y_n = ytnp.tile([128, NG, HD], BF16, name="y_n", tag="y_n")
                if USE_BCAST:
                    nc.vector.tensor_tensor(
                        y_n[:],
                        y_t[:, :, 0:HD],
                        rcp[:].broadcast_to([128, NG, HD]),
                        mybir.AluOpType.mult,
                    )
                else:
                    for hi in range(4):
                        nc.vector.tensor_scalar_mul(
                            y_n[:, hi, :], y_t[:, hi, 0:HD], rcp[:, hi, :]
                        )
                return comb, y_n

            def attn_tr(g, qb, comb, y_n):
                """transpose back to head-major (bf16 psum region of comb)."""
                if USE_IS_TRANSPOSE:
                    yTp = comb[:, 260:388].bitcast(BF16).rearrange("p (m q) -> p m q", m=2)
                    for mi in range(2):
                        nc.tensor.matmul(
                            yTp[:, mi, :],
                            y_n[:, 2 * mi : 2 * mi + 2, :],
                            id_sb[:],
                            start=True,
                            stop=True,
                            is_transpose=True,
                        )
                else:
                    yTf = pj.tile([128, 512], F32, name="pp", tag="pj")
                    yTp = yTf[:, 0:256].rearrange("p (m q) -> p m q", m=2)
                    for mi in range(2):
                        nc.tensor.matmul(
                            yTp[:, mi, :],
                            y_n[:, 2 * mi : 2 * mi + 2, :],
                            id_sb[:],
                            start=True,
                            stop=True,
                        )
                nc.vector.tensor_copy(
                    yn[:, 2 * g : 2 * g + 2, qb * 128 : qb * 128 + 128], yTp[:]
                )

            ob_cur = [None]

            def out_chunk(tt, n2, borrow=False):
                ob = obp.tile([128, 512], BF16, name="ob", tag="ob")
                if borrow:
                    bt = scp_pool.tile([128, 3, NG, 128], F32, name="scp", tag="sc")
                    op = bt[:, 0, :, :].rearrange("p g q -> p (g q)")
                else:
                    op = pj.tile([128, 512], F32, name="op", tag="pj")
                for m in range(KC):
                    nc.tensor.matmul(
                        op[:],
                        yn[:, m, tt * 128 : (tt + 1) * 128],
                        wo_sb[:, m, n2 * 512 : (n2 + 1) * 512],
                        start=(m == 0),
                        stop=(m == KC - 1),
                    )
                nc.scalar.copy(ob[:], op[:])
                nc.sync.dma_start(
                    out=out[tt * 128 : (tt + 1) * 128, n2 * 512 : (n2 + 1) * 512],
                    in_=ob[:],
                )

            # ---- schedule ----
            # unit (g, qb) pipeline with lag-1 av and lag-2 transpose;
            # proj chunks as filler at the head of each unit (they absorb the
            # exp->scores psum-reuse wait).
            # mid-fillers sit between av(prev) and tr(prev) to absorb the
            # DVE recip/norm latency with PE work.
            mid = {
                (1, 1): [lambda: out_chunk(0, 0)],
                (2, 1): [lambda: out_chunk(0, 1)],
                (1, 2): [lambda: out_chunk(1, 0)],
                (2, 2): [lambda: out_chunk(1, 1)],
                (1, 3): [lambda: out_chunk(2, 0)],
                (2, 3): [lambda: out_chunk(2, 1)],
            }
            pend = None  # (g, qb, ex) awaiting av+tr

            # unit pipeline: scores(u) | av(prev)+recip/norm | masks(u) |
            # filler | transpose(prev). qb0 interleaves the q/v projections;
            # qb>=1 uses out-proj chunks as mid fillers.
            mid_f = {
                (1, 1): [lambda: out_chunk(0, 0)],
                (2, 1): [lambda: out_chunk(0, 1)],
                (1, 2): [lambda: out_chunk(1, 0)],
                (2, 2): [lambda: out_chunk(1, 1)],
                (1, 3): [lambda: out_chunk(2, 0)],
                (2, 3): [lambda: out_chunk(2, 1)],
            }
            PHASE = int(os.environ.get("KV2_PHASE", "5"))
            k_proj(0, 0)
            k_proj(0, 1)
            v_proj(0)
            v_proj(1)
            v_proj(2)
            k_proj(1, 0)
            k_proj(1, 1)
            q_proj(0)
            q_proj(1)
            if PHASE == 1:
                for m in range(2, KC):
                    q_proj(m)
                nc.sync.dma_start(out=out[0:128, 0:512], in_=qT_sb[0][:])
            for g in (range(NG) if PHASE >= 2 else []):
                ex = attn_scores(g, 0)
                if g < 3:
                    q_proj(2 * g + 2)
                if PHASE >= 3 and pend is not None:
                    comb, y_n = attn_av(*pend)
                    attn_masks(g, 0, ex)
                    if g < 3:
                        q_proj(2 * g + 3)
                    v_proj(g + 2)
                    attn_tr(pend[0], pend[1], comb, y_n)
                else:
                    attn_masks(g, 0, ex)
                    if g == 0:
                        q_proj(3)
                    elif g < 3:
                        q_proj(2 * g + 3)
                        v_proj(g + 2)
                pend = (g, 0, ex)
            if PHASE == 2:
                nc.sync.dma_start(out=out[0:128, 0:512], in_=pend[2][:, 0, :, :].rearrange("p a b -> p (a b)"))
            EARLY[0] = False
            if PHASE == 3:
                comb, y_n = attn_av(*pend)
                attn_tr(pend[0], pend[1], comb, y_n)
                nc.sync.dma_start(out=out[0:128, 0:512], in_=yn[:, 0, :])
            for qb in (range(1, 4) if PHASE >= 4 else []):
                for g in range(NG):
                    u = (g, qb)
                    ex = attn_scores(*u)
                    comb, y_n = attn_av(*pend)
                    attn_masks(u[0], u[1], ex)
                    if PHASE >= 5:
                        for f in mid_f.get(u, []):
                            f()
                    attn_tr(pend[0], pend[1], comb, y_n)
                    pend = (u[0], u[1], ex)
            if PHASE >= 4:
                comb, y_n = attn_av(*pend)
            if PHASE >= 5:
                # partial out(3,0) over groups 0-2 (their qb3 tails are done)
                # accumulates in a borrowed scores bank while the last tail's
                # DVE/ScalarE chain drains; heads 6-7 finish after the tr.
                bt30 = scp_pool.tile([128, 3, NG, 128], F32, name="scp", tag="sc")
                op30 = bt30[:, 0, :, :].rearrange("p g q -> p (g q)")
                for m in range(6):
                    nc.tensor.matmul(
                        op30[:],
                        yn[:, m, 384:512],
                        wo_sb[:, m, 0:512],
                        start=(m == 0),
                        stop=False,
                    )
            if PHASE >= 4:
                attn_tr(pend[0], pend[1], comb, y_n)
            if PHASE == 4:
                nc.sync.dma_start(out=out[0:128, 0:512], in_=yn[:, 0, :])
            if PHASE >= 5:
                for m in (6, 7):
                    nc.tensor.matmul(
                        op30[:],
                        yn[:, m, 384:512],
                        wo_sb[:, m, 0:512],
                        start=False,
                        stop=(m == 7),
                    )
                ob30 = obp.tile([128, 512], BF16, name="ob", tag="ob")
                nc.scalar.copy(ob30[:], op30[:])
                nc.sync.dma_start(out=out[384:512, 0:512], in_=ob30[:])
                out_chunk(3, 1, borrow=True)

    _split_multi_waits(nc)
    return nc


_NC = None


def _get_nc():
    global _NC
    if _NC is None:
        _NC = _build_program()
    return _NC


def _to_bf16(a):
    import ml_dtypes

    return np.asarray(a, np.float32).astype(ml_dtypes.bfloat16)


def _to_fp8(a):
    from concourse import mybir as _mb

    return np.asarray(a, np.float32).astype(_mb.dt.np(_mb.dt.float8e4))


def _host_prep(x, Wq, bq, Wk, bk, Wv, bv, Wo, bo):
    x = np.asarray(x, np.float32)
    Wq = np.asarray(Wq, np.float32)
    bq = np.asarray(bq, np.float32)
    Wk = np.asarray(Wk, np.float32)
    bk = np.asarray(bk, np.float32)
    Wv = np.asarray(Wv, np.float32)
    bv = np.asarray(bv, np.float32)
    Wo = np.asarray(Wo, np.float32)
    bo = np.asarray(bo, np.float32)

    scale = np.float32(1.0 / np.sqrt(HD))
    # wq[m][p, kc, n] = Wq[kc*128+p, m*128+n] * scale
    wq_t = _to_bf16(
        np.ascontiguousarray((Wq * scale).reshape(KC, 128, KC, 128).transpose(2, 1, 0, 3))
    )
    wk_t = _to_bf16(np.ascontiguousarray(Wk.reshape(KC, 128, KV).transpose(1, 0, 2)))
    wv_t = _to_bf16(np.ascontiguousarray(Wv.reshape(KC, 128, KV).transpose(1, 0, 2)))
    wo_t = _to_bf16(np.ascontiguousarray(Wo.reshape(KC, 128, C).transpose(1, 0, 2)))
    # bq_sb[p, m] = bq[m*128+p] * scale
    bq_t = np.ascontiguousarray((bq * scale).reshape(KC, 128).T)
    # bk_sb[gh*64+d, mt] = bk[(2*mt+gh)*64+d]
    bk_t = np.ascontiguousarray(bk.reshape(2, 2, 64).transpose(1, 2, 0).reshape(128, 2))
    ident = _to_bf16(np.eye(128, dtype=np.float32))

    # masks [128 kj, slot, g(replicated), 128 qi]
    kj = np.arange(128)[:, None]
    qi = np.arange(128)[None, :]
    t0 = (kj >= qi).astype(np.float32)  # kt0 (and qb0-interior)
    t2 = (kj <= qi).astype(np.float32)  # kt2
    ones = np.ones((128, 128), np.float32)
    zeros = np.zeros((128, 128), np.float32)

    def mk_mask(edge):
        # slots: 0 = kt0@qb0, 1 = kt1@qb0, 2 = kt0@qb>0, 3 = kt2
        s0 = zeros if edge else t0
        s1 = zeros if edge else ones
        m = np.stack([s0, s1, t0, t2])  # [4, 128, 128]
        return _to_bf16(np.broadcast_to(m[None, :, :, :], (NG, 4, 128, 128)).transpose(2, 1, 0, 3).copy())

    mask_std = mk_mask(False)
    mask_edge = mk_mask(True)

    in_maps = []
    for core in range(NCORES):
        b, c = core // 4, core % 4
        t0c = c * TQ - WINDOW
        xe = np.zeros((TE, C), np.float32)
        lo = max(t0c, 0)
        xe[lo - t0c : TE, :] = x[b, lo : t0c + TE, :]
        xt_t = _to_bf16(np.ascontiguousarray(xe.T.reshape(KC, 128, TE).transpose(1, 0, 2)))
        in_maps.append(
            {
                "xt": xt_t,
                "wq": wq_t,
                "wk": wk_t,
                "wv": wv_t,
                "wo": wo_t,
                "bq": bq_t,
                "bk": bk_t,
                "ident": ident,
                "maskp": mask_edge if c == 0 else mask_std,
            }
        )

    bv_rep = np.concatenate([bv[(h // NG) * HD : (h // NG + 1) * HD] for h in range(NH)])
    corr = bv_rep.astype(np.float64) @ Wo.astype(np.float64) + bo.astype(np.float64)
    return in_maps, corr.astype(np.float32)


LAST_RESULTS = None


def kernel(x, Wq, bq, Wk, bk, Wv, bv, Wo, bo):
    global LAST_RESULTS
    in_maps, corr = _host_prep(x, Wq, bq, Wk, bk, Wv, bv, Wo, bo)
    nc = _get_nc()
    res = run_bass_kernel_spmd(nc, in_maps, core_ids=list(range(NCORES)))
    LAST_RESULTS = res
    out = np.empty((B, T, C), np.float32)
    for core in range(NCORES):
        b, c = core // 4, core % 4
        out[b, c * TQ : (c + 1) * TQ, :] = res.results[core]["out"].astype(np.float32)
    out += corr[None, None, :]
    return out


# revision 9
# speedup vs baseline: 1.0007x; 1.0007x over previous
"""Trainium2 Bass kernel for GQA causal sliding-window self-attention.

Problem: B=2, T=2048, C=1024, 16 heads (hd=64), 4 KV groups, window=256.

Sharding: data-parallel over (batch, T-chunk): 8 cores = 2 batches x 4
chunks of 512 query tokens; each core gets a 768-token extended x slice
(512 queries + 256 halo) and computes its output rows locally.

Design: all-bf16 operands (halves DMA, 1 cyc/col matmuls at any free
size), diagonal 128-query attention blocks (384-key span = 1.5x
overcompute instead of 2x), 2-head-packed score matmuls (qT stored
[64, 2, TQ] per head-pair, kT [64, NG, TE], all matmul operands at base
partition 0), TRANSPOSED av (queries on partitions; ones column in v
gives softmax denominators) so the reciprocal is per-partition and
normalization is one stride-0-broadcast DVE multiply, then a transpose
back to head-major via a plain bf16 matmul against an identity matrix.
Band masks are multiplicative 0/1 bf16 constants (GPSIMD kt0 +
qb0-edge kt1; DVE kt2; the middle kt tile needs no mask), applied to
the exp'd scores. Output is bf16, upcast on host; host folds 1/8 into
Wq/bq and adds the exact linear bv/bo correction (bv_rep @ Wo + bo).

PSUM (8 banks): scores [128,3,4,128] x2 bufs (6), combined
y_t[128,4,65]+bf16 transpose region x1 (1), shared proj/out bank (1).
Early projections borrow the scores pool before attention starts.

Pipeline: per unit (group, 128-query block): scores -> exp (ScalarE)
-> masks -> av -> recip/norm (DVE) -> transpose -> yn copy, with
q/v-projection and out-projection chunks interleaved as PE filler.

Environment constraints found the hard way (this walrus/axon build):
 - max 1 sync wait per instruction (_split_multi_waits hoists extras
   onto same-engine NOPs)
 - bf16 matmuls with operands at base partition 64 crash the compile;
   all operands must sit at partition 0 (hence qT/kT layouts)
 - is_transpose=True matmuls return wrong data on hw (interp-only)
 - only one open PSUM accumulation group per 2KB bank
 - GPSIMD cannot touch PSUM; DVE/ScalarE cross-partition-offset ok
 - DMA cannot read PSUM
"""

import sys

sys.path.insert(0, "/opt/trn_rl_repo")

import contextlib

import numpy as np

import concourse.bass as bass
import concourse.tile as tile
from concourse import mybir
from concourse.bass_utils import run_bass_kernel_spmd
from concourse.vector_clock import ScopedClock

F32 = mybir.dt.float32
BF16 = mybir.dt.bfloat16
F8E4 = mybir.dt.float8e4
FP8_SCALE = 32.0  # weights *32 into fp8 normal range; q carries the 1/32

import os

USE_IS_TRANSPOSE = os.environ.get("KV2_ISTR", "") == "1"  # wrong results on hw
USE_BCAST = os.environ.get("KV2_NO_BCAST", "") != "1"
USE_SC_IDBIAS = os.environ.get("KV2_NO_IDBIAS", "") != "1"

B, T, C = 2, 2048, 1024
NH, NG, HD = 16, 4, 64
KV = NG * HD  # 256
WINDOW = 256
NCORES = 8
TQ = 512  # query tokens per core
TE = TQ + WINDOW  # 768 extended tokens per core
KC = C // 128  # 8 contraction tiles


class _ChunkedDrainTileContext(tile.TileContext):
    """Walrus in this container only accepts 1 sync wait on CTRL-class
    instructions; spread the tail drain waits over engine NOPs."""

    def _drain_and_barrier(self, tick_clock, wait_clock):
        gc = tick_clock.global_clock
        entries = []
        for scope, vc in ScopedClock({None: gc}).items():
            for proc in range(len(vc)):
                t = vc[proc]
                if t > 0:
                    entries.append((scope, proc, t))
        engines = [self.nc.sync, self.nc.vector, self.nc.scalar, self.nc.gpsimd]
        curs = [ScopedClock() for _ in engines]
        for i, (scope, proc, t) in enumerate(entries):
            eng = engines[i % len(engines)]
            nop = eng.nop(nofuse=True, hint="tail_wait")
            partial = ScopedClock()
            partial.require_at_least(scope, proc, t)
            wait_clock.add_sem_waits(nop.ins, partial, curs[i % len(engines)])
            curs[i % len(engines)].update_past(partial)
        self.nc.all_engine_barrier(sem_only=True)
        drain_inst = self.nc.sync.drain()
        cur = ScopedClock()
        for c in curs:
            cur.update_past(c)
        wait_clock.add_sem_waits(drain_inst.ins, ScopedClock({None: gc}), cur)
        assert self.sems is not None
        popped = self.nc._tile_sem_poison_stack.pop()
        assert popped is self._sem_poison
        self.nc.clear_and_free_semaphores(list(self.sems.allocated().values()))


def _split_multi_waits(nc, max_waits=1):
    """Hoist excess sync waits onto same-engine NOPs (walrus quirk)."""
    fn = nc.m.functions[0]
    for blk in fn.blocks:
        insts = blk.instructions
        new = []
        changed = False
        for inst in insts:
            si = inst.sync_info
            waits = list(si.on_wait) if si is not None and si.on_wait else []
            if len(waits) > max_waits:
                changed = True
                for w in waits[:-max_waits]:
                    nop = mybir.InstNoOp(
                        name=nc.get_next_instruction_name(),
                        ins=[],
                        outs=[],
                        engine=inst.engine,
                        sync_info=mybir.SyncInfo(on_wait=[w], on_update=[]),
                        bass_nofuse=True,
                    )
                    nc.register_instruction(nop, overwrite=True)
                    new.append(nop)
                si.on_wait = waits[-max_waits:]
                inst.sync_info = si
            new.append(inst)
        if changed:
            blk.instructions = new


def _build_program():
    nc = bass.Bass("TRN2", target_bir_lowering=False, debug=False, num_devices=NCORES)

    xt = nc.dram_tensor("xt", [128, KC, TE], BF16, kind="ExternalInput")
    wq = nc.dram_tensor("wq", [KC, 128, KC, 128], BF16, kind="ExternalInput")
    wk = nc.dram_tensor("wk", [128, KC, KV], BF16, kind="ExternalInput")
    wv = nc.dram_tensor("wv", [128, KC, KV], BF16, kind="ExternalInput")
    wo = nc.dram_tensor("wo", [128, KC, C], BF16, kind="ExternalInput")
    bq = nc.dram_tensor("bq", [128, KC], F32, kind="ExternalInput")
    bk = nc.dram_tensor("bk", [128, 2], F32, kind="ExternalInput")
    ident = nc.dram_tensor("ident", [128, 128], BF16, kind="ExternalInput")
    # mask slots: 0 = kt0@qb0, 1 = kt1@qb0, 2 = kt0@qb>0 (T0), 3 = kt2 (T2)
    maskp = nc.dram_tensor("maskp", [128, 4, NG, 128], BF16, kind="ExternalInput")
    out = nc.dram_tensor("out", [TQ, C], BF16, kind="ExternalOutput")

    with _ChunkedDrainTileContext(nc) as tc:
        with contextlib.ExitStack() as ctx:
            wsb = ctx.enter_context(tc.tile_pool(name="wsb", bufs=1))
            xsb = ctx.enter_context(tc.tile_pool(name="xsb", bufs=1))
            csb = ctx.enter_context(tc.tile_pool(name="csb", bufs=1))
            qkv = ctx.enter_context(tc.tile_pool(name="qkv", bufs=1))
            ynp = ctx.enter_context(tc.tile_pool(name="ynp", bufs=1))
            expp = ctx.enter_context(tc.tile_pool(name="expp", bufs=8))
            rcpp = ctx.enter_context(tc.tile_pool(name="rcpp", bufs=4))
            ytnp = ctx.enter_context(tc.tile_pool(name="ytnp", bufs=6))
            obp = ctx.enter_context(tc.tile_pool(name="obp", bufs=3))
            pj = ctx.enter_context(tc.tile_pool(name="pj", bufs=1, space="PSUM"))
            # av-output y_t [128,4,65] gets its own bank
            ytp_pool = ctx.enter_context(tc.tile_pool(name="ytp", bufs=1, space="PSUM"))
            scp_pool = ctx.enter_context(tc.tile_pool(name="scp", bufs=2, space="PSUM"))

            EARLY = [True]

            def proj_psum():
                # before the attention pipeline starts, projections borrow
                # the scores pool's big tiles (double-buffered); once units
                # are flowing they use the dedicated pj bank so they don't
                # steal the scores rotation.
                if EARLY[0]:
                    t = scp_pool.tile([128, 3, NG, 128], F32, name="scp", tag="sc")
                    return t[:, 0, :, :].rearrange("p g q -> p (g q)")
                return pj.tile([128, 512], F32, name="pp", tag="pj")

            # ---- loads, ordered by consumption deadline ----
            wk_sb = wsb.tile([128, KC, KV], BF16, name="wk_sb", tag="wk")
            nc.sync.dma_start(out=wk_sb[:, 0:4, 0:128], in_=wk[:, 0:4, 0:128])
            xt_sb = xsb.tile([128, KC, TE], BF16, name="xt_sb", tag="xt")
            nc.sync.dma_start(out=xt_sb[:, 0:2, 0:384], in_=xt[:, 0:2, 0:384])
            nc.sync.dma_start(out=xt_sb[:, 2:4, 0:384], in_=xt[:, 2:4, 0:384])
            nc.sync.dma_start(out=wk_sb[:, 4:8, 0:128], in_=wk[:, 4:8, 0:128])
            nc.sync.dma_start(out=xt_sb[:, 4:6, 0:384], in_=xt[:, 4:6, 0:384])
            nc.sync.dma_start(out=xt_sb[:, 6:8, 0:384], in_=xt[:, 6:8, 0:384])
            nc.sync.dma_start(out=wk_sb[:, :, 128:256], in_=wk[:, :, 128:256])
            bk_sb = csb.tile([128, 2], F32)
            nc.sync.dma_start(out=bk_sb[:], in_=bk[:])
            wv_sb = wsb.tile([128, KC, KV], BF16, name="wv_sb", tag="wv")
            nc.sync.dma_start(out=wv_sb[:], in_=wv[:])
            nc.sync.dma_start(out=xt_sb[:, 0:4, 384:TE], in_=xt[:, 0:4, 384:TE])
            nc.sync.dma_start(out=xt_sb[:, 4:8, 384:TE], in_=xt[:, 4:8, 384:TE])
            bq_sb = csb.tile([128, KC], F32)
            nc.sync.dma_start(out=bq_sb[:], in_=bq[:])

            wq_sb = [None] * KC

            def _load_wq(m):
                t = wsb.tile([128, KC, 128], BF16, name=f"wq{m}", tag=f"wq{m}")
                nc.sync.dma_start(out=t[:], in_=wq[m, :, :, :])
                wq_sb[m] = t

            for m in range(2):
                _load_wq(m)
            id_sb = csb.tile([128, 128], BF16)
            nc.sync.dma_start(out=id_sb[:], in_=ident[:])
            mask_sb = csb.tile([128, 4, NG, 128], BF16)
            nc.sync.dma_start(out=mask_sb[:], in_=maskp[:])
            for m in range(2, KC):
                _load_wq(m)
            wo_sb = wsb.tile([128, KC, C], BF16, name="wo_sb", tag="wo")
            nc.sync.dma_start(out=wo_sb[:, :, 0:512], in_=wo[:, :, 0:512])
            nc.sync.dma_start(out=wo_sb[:, :, 512:1024], in_=wo[:, :, 512:1024])

            # ---- kT projection: kT4 [64, NG, TE] bf16 (base-partition 0;
            # bf16 matmuls with operands at base partition 64 crash walrus)
            kT4 = qkv.tile([64, NG, TE], BF16, name="kT4", tag="kT4")

            def k_proj(s2, mt):
                kp = proj_psum()
                for kc in range(KC):
                    nc.tensor.matmul(
                        kp[:, 0:384],
                        wk_sb[:, kc, mt * 128 : (mt + 1) * 128],
                        xt_sb[:, kc, s2 * 384 : (s2 + 1) * 384],
                        start=(kc == 0),
                        stop=(kc == KC - 1),
                    )
                for gh in range(2):
                    g = 2 * mt + gh
                    if gh == 0:
                        nc.vector.tensor_scalar_add(
                            kT4[0:64, g, s2 * 384 : (s2 + 1) * 384],
                            kp[0:64, 0:384],
                            bk_sb[0:64, mt : mt + 1],
                        )
                    else:
                        nc.scalar.activation(
                            kT4[0:64, g, s2 * 384 : (s2 + 1) * 384],
                            kp[64:128, 0:384],
                            mybir.ActivationFunctionType.Identity,
                            bias=bk_sb[64:128, mt : mt + 1],
                        )

            # ---- v projection: token-major [128, NG, 65] with ones column ----
            v_sb = []
            for vt in range(6):
                t = qkv.tile([128, NG, HD + 1], BF16, name=f"v{vt}", tag=f"v{vt}")
                nc.vector.memset(t[:, :, HD : HD + 1], 1.0)
                v_sb.append(t)

            def v_proj(vt):
                vp = proj_psum()
                for kc in range(KC):
                    nc.tensor.matmul(
                        vp[:, 0:KV],
                        xt_sb[:, kc, vt * 128 : (vt + 1) * 128],
                        wv_sb[:, kc, :],
                        start=(kc == 0),
                        stop=(kc == KC - 1),
                    )
                nc.scalar.copy(
                    v_sb[vt][:, :, 0:HD],
                    vp[:, 0:KV].rearrange("p (g d) -> p g d", g=NG),
                )

            yn = ynp.tile([128, KC, TQ], BF16)
            qT_sb = [None] * KC

            def q_proj(m):
                qp = proj_psum()
                for kc in range(KC):
                    nc.tensor.matmul(
                        qp[:],
                        wq_sb[m][:, kc, :],
                        xt_sb[:, kc, WINDOW:TE],
                        start=(kc == 0),
                        stop=(kc == KC - 1),
                    )
                qT = qkv.tile([64, 2, TQ], BF16, name=f"qT{m}", tag=f"qT{m}")
                nc.vector.tensor_scalar_add(
                    qT[0:64, 0, :], qp[0:64, :], bq_sb[0:64, m : m + 1]
                )
                nc.scalar.activation(
                    qT[0:64, 1, :],
                    qp[64:128, :],
                    mybir.ActivationFunctionType.Identity,
                    bias=bq_sb[64:128, m : m + 1],
                )
                qT_sb[m] = qT

            def attn_scores(g, qb):
                """12 score matmuls + exp + masks; returns masked ex tile."""
                scp = scp_pool.tile([128, 3, NG, 128], F32, name="scp", tag="sc")
                for kt in range(3):
                    ke0 = qb * 128 + kt * 128
                    for mi in range(2):
                        m = 2 * g + mi
                        # 2 heads per matmul: rhs [64, 2, 128] (hi = 2*mi+hh)
                        nc.tensor.matmul(
                            scp[:, kt, 2 * mi : 2 * mi + 2, :],
                            kT4[0:64, g, ke0 : ke0 + 128],
                            qT_sb[m][0:64, :, qb * 128 : qb * 128 + 128],
                            start=True,
                            stop=True,
                        )
                ex = expp.tile([128, 3, NG, 128], BF16, name="ex", tag="ex")
                nc.scalar.activation(ex[:], scp[:], mybir.ActivationFunctionType.Exp)
                return ex

            def attn_masks(g, qb, ex):
                # band masks (multiplicative 0/1): kt0 always, kt1 only at qb0,
                # kt2 always. Middle tile fully valid for qb>0. Emitted AFTER
                # the previous unit's recip/norm so they don't head-of-line
                # block the DVE queue while waiting on exp.
                # edge cores: qb0 kt0/kt1 fully padded; qb1 kt0 also reaches
                # into the padding (slot 0 = T0 on std cores, zeros on edge)
                slot0 = 0 if qb <= 1 else 2
                nc.vector.tensor_tensor(
                    ex[:, 0, :, :], ex[:, 0, :, :], mask_sb[:, slot0, :, :],
                    mybir.AluOpType.mult,
                )
                if qb == 0:
                    nc.gpsimd.tensor_tensor(
                        ex[:, 1, :, :], ex[:, 1, :, :], mask_sb[:, 1, :, :],
                        mybir.AluOpType.mult,
                    )
                nc.vector.tensor_tensor(
                    ex[:, 2, :, :], ex[:, 2, :, :], mask_sb[:, 3, :, :],
                    mybir.AluOpType.mult,
                )

            def attn_av(g, qb, ex):
                """av (transposed) + recip + norm. kt order 1,2,0 so the
                unmasked middle tile starts immediately after exp and the
                slow gpsimd kt0 mask gets maximal slack."""
                comb = ytp_pool.tile([128, 388], F32, name="comb", tag="yt")
                y_t = comb[:, 0:260].rearrange("p (h d) -> p h d", h=NG)
                # hi outer: only one psum accumulation group open per bank
                for hi in range(4):
                    for kt in (1, 0, 2):
                        nc.tensor.matmul(
                            y_t[:, hi, :],
                            ex[:, kt, hi, :],
                            v_sb[qb + kt][:, g, :],
                            start=(kt == 1),
                            stop=(kt == 2),
                        )

                rcp = rcpp.tile([128, NG, 1], F32, name="rcp", tag="rcp")
                with nc.allow_low_precision(reason="softmax denom reciprocal"):
                    nc.vector.reciprocal(rcp[:], y_t[:, :, HD : HD + 1])
                y_n = ytnp.tile([128, NG, HD], BF16, name="y_n", tag="y_n")
                if USE_BCAST:
                    nc.vector.tensor_tensor(
                        y_n[:],
                        y_t[:, :, 0:HD],
                        rcp[:].broadcast_to([128, NG, HD]),
                        mybir.AluOpType.mult,
                    )
                else:
                    for hi in range(4):
                        nc.vector.tensor_scalar_mul(
                            y_n[:, hi, :], y_t[:, hi, 0:HD], rcp[:, hi, :]
                        )
                return comb, y_n

            def attn_tr(g, qb, comb, y_n):
                """transpose back to head-major (bf16 psum region of comb)."""
                if USE_IS_TRANSPOSE:
                    yTp = comb[:, 260:388].bitcast(BF16).rearrange("p (m q) -> p m q", m=2)
                    for mi in range(2):
                        nc.tensor.matmul(
                            yTp[:, mi, :],
                            y_n[:, 2 * mi : 2 * mi + 2, :],
                            id_sb[:],
                            start=True,
                            stop=True,
                            is_transpose=True,
                        )
                else:
                    yTf = pj.tile([128, 512], F32, name="pp", tag="pj")
                    yTp = yTf[:, 0:256].rearrange("p (m q) -> p m q", m=2)
                    for mi in range(2):
                        nc.tensor.matmul(
                            yTp[:, mi, :],
                            y_n[:, 2 * mi : 2 * mi + 2, :],
                            id_sb[:],
                            start=True,
                            stop=True,
                        )
                nc.vector.tensor_copy(
                    yn[:, 2 * g : 2 * g + 2, qb * 128 : qb * 128 + 128], yTp[:]
                )

            ob_cur = [None]

            def out_chunk(tt, n2, borrow=False):
                ob = obp.tile([128, 512], BF16, name="ob", tag="ob")
                if borrow:
                    bt = scp_pool.tile([128, 3, NG, 128], F32, name="scp", tag="sc")
                    op = bt[:, 0, :, :].rearrange("p g q -> p (g q)")
                else:
                    op = pj.tile([128, 512], F32, name="op", tag="pj")
                for m in range(KC):
                    nc.tensor.matmul(
                        op[:],
                        yn[:, m, tt * 128 : (tt + 1) * 128],
                        wo_sb[:, m, n2 * 512 : (n2 + 1) * 512],
                        start=(m == 0),
                        stop=(m == KC - 1),
                    )
                nc.scalar.copy(ob[:], op[:])
                nc.sync.dma_start(
                    out=out[tt * 128 : (tt + 1) * 128, n2 * 512 : (n2 + 1) * 512],
                    in_=ob[:],
                )

            # ---- schedule ----
            # unit (g, qb) pipeline with lag-1 av and lag-2 transpose;
            # proj chunks as filler at the head of each unit (they absorb the
            # exp->scores psum-reuse wait).
            # mid-fillers sit between av(prev) and tr(prev) to absorb the
            # DVE recip/norm latency with PE work.
            mid = {
                (1, 1): [lambda: out_chunk(0, 0)],
                (2, 1): [lambda: out_chunk(0, 1)],
                (1, 2): [lambda: out_chunk(1, 0)],
                (2, 2): [lambda: out_chunk(1, 1)],
                (1, 3): [lambda: out_chunk(2, 0)],
                (2, 3): [lambda: out_chunk(2, 1)],
            }
            pend = None  # (g, qb, ex) awaiting av+tr

            # unit pipeline: scores(u) | av(prev)+recip/norm | masks(u) |
            # filler | transpose(prev). qb0 interleaves the q/v projections;
            # qb>=1 uses out-proj chunks as mid fillers.
            mid_f = {
                (1, 1): [lambda: out_chunk(0, 0)],
                (2, 1): [lambda: out_chunk(0, 1)],
                (1, 2): [lambda: out_chunk(1, 0)],
                (2, 2): [lambda: out_chunk(1, 1)],
                (1, 3): [lambda: out_chunk(2, 0)],
                (2, 3): [lambda: out_chunk(2, 1)],
            }
            PHASE = int(os.environ.get("KV2_PHASE", "5"))
            k_proj(0, 0)
            k_proj(0, 1)
            v_proj(0)
            v_proj(1)
            v_proj(2)
            k_proj(1, 0)
            k_proj(1, 1)
            q_proj(0)
            q_proj(1)
            if PHASE == 1:
                for m in range(2, KC):
                    q_proj(m)
                nc.sync.dma_start(out=out[0:128, 0:512], in_=qT_sb[0][:])
            for g in (range(NG) if PHASE >= 2 else []):
                ex = attn_scores(g, 0)
                if g < 3:
                    q_proj(2 * g + 2)
                if PHASE >= 3 and pend is not None:
                    comb, y_n = attn_av(*pend)
                    attn_masks(g, 0, ex)
                    if g < 3:
                        q_proj(2 * g + 3)
                    v_proj(g + 2)
                    attn_tr(pend[0], pend[1], comb, y_n)
                else:
                    attn_masks(g, 0, ex)
                    if g == 0:
                        q_proj(3)
                    elif g < 3:
                        q_proj(2 * g + 3)
                        v_proj(g + 2)
                pend = (g, 0, ex)
            if PHASE == 2:
                nc.sync.dma_start(out=out[0:128, 0:512], in_=pend[2][:, 0, :, :].rearrange("p a b -> p (a b)"))
            EARLY[0] = False
            if PHASE == 3:
                comb, y_n = attn_av(*pend)
                attn_tr(pend[0], pend[1], comb, y_n)
                nc.sync.dma_start(out=out[0:128, 0:512], in_=yn[:, 0, :])
            for qb in (range(1, 4) if PHASE >= 4 else []):
                for g in range(NG):
                    u = (g, qb)
                    ex = attn_scores(*u)
                    comb, y_n = attn_av(*pend)
                    attn_masks(u[0], u[1], ex)
                    if PHASE >= 5:
                        for f in mid_f.get(u, []):
                            f()
                    attn_tr(pend[0], pend[1], comb, y_n)
                    pend = (u[0], u[1], ex)
            if PHASE >= 4:
                comb, y_n = attn_av(*pend)
            if PHASE >= 5:
                # partial out(3,0) over groups 0-2 (their qb3 tails are done)
                # accumulates in a borrowed scores bank while the last tail's
                # DVE/ScalarE chain drains; heads 6-7 finish after the tr.
                bt30 = scp_pool.tile([128, 3, NG, 128], F32, name="scp", tag="sc")
                op30 = bt30[:, 0, :, :].rearrange("p g q -> p (g q)")
                for m in range(6):
                    nc.tensor.matmul(
                        op30[:],
                        yn[:, m, 384:512],
                        wo_sb[:, m, 0:512],
                        start=(m == 0),
                        stop=False,
                    )
            if PHASE >= 4:
                attn_tr(pend[0], pend[1], comb, y_n)
            if PHASE == 4:
                nc.sync.dma_start(out=out[0:128, 0:512], in_=yn[:, 0, :])
            if PHASE >= 5:
                for m in (6, 7):
                    nc.tensor.matmul(
                        op30[:],
                        yn[:, m, 384:512],
                        wo_sb[:, m, 0:512],
                        start=False,
                        stop=(m == 7),
                    )
                ob30 = obp.tile([128, 512], BF16, name="ob", tag="ob")
                nc.scalar.copy(ob30[:], op30[:])
                nc.sync.dma_start(out=out[384:512, 0:512], in_=ob30[:])
                out_chunk(3, 1, borrow=True)

    _split_multi_waits(nc)
    return nc


_NC = None


def _get_nc():
    global _NC
    if _NC is None:
        _NC = _build_program()
    return _NC


def _to_bf16(a):
    import ml_dtypes

    return np.asarray(a, np.float32).astype(ml_dtypes.bfloat16)


def _to_fp8(a):
    from concourse import mybir as _mb

    return np.asarray(a, np.float32).astype(_mb.dt.np(_mb.dt.float8e4))


def _host_prep(x, Wq, bq, Wk, bk, Wv, bv, Wo, bo):
    x = np.asarray(x, np.float32)
    Wq = np.asarray(Wq, np.float32)
    bq = np.asarray(bq, np.float32)
    Wk = np.asarray(Wk, np.float32)
    bk = np.asarray(bk, np.float32)
    Wv = np.asarray(Wv, np.float32)
    bv = np.asarray(bv, np.float32)
    Wo = np.asarray(Wo, np.float32)
    bo = np.asarray(bo, np.float32)

    scale = np.float32(1.0 / np.sqrt(HD))
    # wq[m][p, kc, n] = Wq[kc*128+p, m*128+n] * scale
    wq_t = _to_bf16(
        np.ascontiguousarray((Wq * scale).reshape(KC, 128, KC, 128).transpose(2, 1, 0, 3))
    )
    wk_t = _to_bf16(np.ascontiguousarray(Wk.reshape(KC, 128, KV).transpose(1, 0, 2)))
    wv_t = _to_bf16(np.ascontiguousarray(Wv.reshape(KC, 128, KV).transpose(1, 0, 2)))
    wo_t = _to_bf16(np.ascontiguousarray(Wo.reshape(KC, 128, C).transpose(1, 0, 2)))
    # bq_sb[p, m] = bq[m*128+p] * scale
    bq_t = np.ascontiguousarray((bq * scale).reshape(KC, 128).T)
    # bk_sb[gh*64+d, mt] = bk[(2*mt+gh)*64+d]
    bk_t = np.ascontiguousarray(bk.reshape(2, 2, 64).transpose(1, 2, 0).reshape(128, 2))
    ident = _to_bf16(np.eye(128, dtype=np.float32))

    # masks [128 kj, slot, g(replicated), 128 qi]
    kj = np.arange(128)[:, None]
    qi = np.arange(128)[None, :]
    t0 = (kj >= qi).astype(np.float32)  # kt0 (and qb0-interior)
    t2 = (kj <= qi).astype(np.float32)  # kt2
    ones = np.ones((128, 128), np.float32)
    zeros = np.zeros((128, 128), np.float32)

    def mk_mask(edge):
        # slots: 0 = kt0@qb0, 1 = kt1@qb0, 2 = kt0@qb>0, 3 = kt2
        s0 = zeros if edge else t0
        s1 = zeros if edge else ones
        m = np.stack([s0, s1, t0, t2])  # [4, 128, 128]
        return _to_bf16(np.broadcast_to(m[None, :, :, :], (NG, 4, 128, 128)).transpose(2, 1, 0, 3).copy())

    mask_std = mk_mask(False)
    mask_edge = mk_mask(True)

    in_maps = []
    for core in range(NCORES):
        b, c = core // 4, core % 4
        t0c = c * TQ - WINDOW
        xe = np.zeros((TE, C), np.float32)
        lo = max(t0c, 0)
        xe[lo - t0c : TE, :] = x[b, lo : t0c + TE, :]
        xt_t = _to_bf16(np.ascontiguousarray(xe.T.reshape(KC, 128, TE).transpose(1, 0, 2)))
        in_maps.append(
            {
                "xt": xt_t,
                "wq": wq_t,
                "wk": wk_t,
                "wv": wv_t,
                "wo": wo_t,
                "bq": bq_t,
                "bk": bk_t,
                "ident": ident,
                "maskp": mask_edge if c == 0 else mask_std,
            }
        )

    bv_rep = np.concatenate([bv[(h // NG) * HD : (h // NG + 1) * HD] for h in range(NH)])
    corr = bv_rep.astype(np.float64) @ Wo.astype(np.float64) + bo.astype(np.float64)
    return in_maps, corr.astype(np.float32)


LAST_RESULTS = None


def kernel(x, Wq, bq, Wk, bk, Wv, bv, Wo, bo):
    global LAST_RESULTS
    in_maps, corr = _host_prep(x, Wq, bq, Wk, bk, Wv, bv, Wo, bo)
    nc = _get_nc()
    res = run_bass_kernel_spmd(nc, in_maps, core_ids=list(range(NCORES)))
    LAST_RESULTS = res
    out = np.empty((B, T, C), np.float32)
    for core in range(NCORES):
        b, c = core // 4, core % 4
        out[b, c * TQ : (c + 1) * TQ, :] = res.results[core]["out"].astype(np.float32)
    out += corr[None, None, :]
    return out


# revision 10
# speedup vs baseline: 1.0034x; 1.0027x over previous
"""Trainium2 Bass kernel for GQA causal sliding-window self-attention.

Problem: B=2, T=2048, C=1024, 16 heads (hd=64), 4 KV groups, window=256.

Sharding: data-parallel over (batch, T-chunk): 8 cores = 2 batches x 4
chunks of 512 query tokens; each core gets a 768-token extended x slice
(512 queries + 256 halo) and computes its output rows locally.

Design: all-bf16 operands (halves DMA, 1 cyc/col matmuls at any free
size), diagonal 128-query attention blocks (384-key span = 1.5x
overcompute instead of 2x), 2-head-packed score matmuls (qT stored
[64, 2, TQ] per head-pair, kT [64, NG, TE], all matmul operands at base
partition 0), TRANSPOSED av (queries on partitions; ones column in v
gives softmax denominators) so the reciprocal is per-partition and
normalization is one stride-0-broadcast DVE multiply, then a transpose
back to head-major via a plain bf16 matmul against an identity matrix.
Band masks are multiplicative 0/1 bf16 constants (GPSIMD kt0 +
qb0-edge kt1; DVE kt2; the middle kt tile needs no mask), applied to
the exp'd scores. Output is bf16, upcast on host; host folds 1/8 into
Wq/bq and adds the exact linear bv/bo correction (bv_rep @ Wo + bo).

PSUM (8 banks): scores [128,3,4,128] x2 bufs (6), combined
y_t[128,4,65]+bf16 transpose region x1 (1), shared proj/out bank (1).
Early projections borrow the scores pool before attention starts.

Pipeline: per unit (group, 128-query block): scores -> exp (ScalarE)
-> masks -> av -> recip/norm (DVE) -> transpose -> yn copy, with
q/v-projection and out-projection chunks interleaved as PE filler.

Environment constraints found the hard way (this walrus/axon build):
 - max 1 sync wait per instruction (_split_multi_waits hoists extras
   onto same-engine NOPs)
 - bf16 matmuls with operands at base partition 64 crash the compile;
   all operands must sit at partition 0 (hence qT/kT layouts)
 - is_transpose=True matmuls return wrong data on hw (interp-only)
 - only one open PSUM accumulation group per 2KB bank
 - GPSIMD cannot touch PSUM; DVE/ScalarE cross-partition-offset ok
 - DMA cannot read PSUM
"""

import sys

sys.path.insert(0, "/opt/trn_rl_repo")

import contextlib

import numpy as np

import concourse.bass as bass
import concourse.tile as tile
from concourse import mybir
from concourse.bass_utils import run_bass_kernel_spmd
from concourse.vector_clock import ScopedClock

F32 = mybir.dt.float32
BF16 = mybir.dt.bfloat16
F8E4 = mybir.dt.float8e4
FP8_SCALE = 32.0  # weights *32 into fp8 normal range; q carries the 1/32

import os

USE_IS_TRANSPOSE = os.environ.get("KV2_ISTR", "") == "1"  # wrong results on hw
USE_BCAST = os.environ.get("KV2_NO_BCAST", "") != "1"
USE_SC_IDBIAS = os.environ.get("KV2_NO_IDBIAS", "") != "1"

B, T, C = 2, 2048, 1024
NH, NG, HD = 16, 4, 64
KV = NG * HD  # 256
WINDOW = 256
NCORES = 8
TQ = 512  # query tokens per core
TE = TQ + WINDOW  # 768 extended tokens per core
KC = C // 128  # 8 contraction tiles


class _ChunkedDrainTileContext(tile.TileContext):
    """Walrus in this container only accepts 1 sync wait on CTRL-class
    instructions; spread the tail drain waits over engine NOPs."""

    def _drain_and_barrier(self, tick_clock, wait_clock):
        gc = tick_clock.global_clock
        entries = []
        for scope, vc in ScopedClock({None: gc}).items():
            for proc in range(len(vc)):
                t = vc[proc]
                if t > 0:
                    entries.append((scope, proc, t))
        engines = [self.nc.sync, self.nc.vector, self.nc.scalar, self.nc.gpsimd]
        curs = [ScopedClock() for _ in engines]
        for i, (scope, proc, t) in enumerate(entries):
            eng = engines[i % len(engines)]
            nop = eng.nop(nofuse=True, hint="tail_wait")
            partial = ScopedClock()
            partial.require_at_least(scope, proc, t)
            wait_clock.add_sem_waits(nop.ins, partial, curs[i % len(engines)])
            curs[i % len(engines)].update_past(partial)
        self.nc.all_engine_barrier(sem_only=True)
        drain_inst = self.nc.sync.drain()
        cur = ScopedClock()
        for c in curs:
            cur.update_past(c)
        wait_clock.add_sem_waits(drain_inst.ins, ScopedClock({None: gc}), cur)
        assert self.sems is not None
        popped = self.nc._tile_sem_poison_stack.pop()
        assert popped is self._sem_poison
        self.nc.clear_and_free_semaphores(list(self.sems.allocated().values()))


def _split_multi_waits(nc, max_waits=1):
    """Hoist excess sync waits onto same-engine NOPs (walrus quirk)."""
    fn = nc.m.functions[0]
    for blk in fn.blocks:
        insts = blk.instructions
        new = []
        changed = False
        for inst in insts:
            si = inst.sync_info
            waits = list(si.on_wait) if si is not None and si.on_wait else []
            if len(waits) > max_waits:
                changed = True
                for w in waits[:-max_waits]:
                    nop = mybir.InstNoOp(
                        name=nc.get_next_instruction_name(),
                        ins=[],
                        outs=[],
                        engine=inst.engine,
                        sync_info=mybir.SyncInfo(on_wait=[w], on_update=[]),
                        bass_nofuse=True,
                    )
                    nc.register_instruction(nop, overwrite=True)
                    new.append(nop)
                si.on_wait = waits[-max_waits:]
                inst.sync_info = si
            new.append(inst)
        if changed:
            blk.instructions = new


def _build_program():
    nc = bass.Bass("TRN2", target_bir_lowering=False, debug=False, num_devices=NCORES)

    xt = nc.dram_tensor("xt", [128, KC, TE], BF16, kind="ExternalInput")
    wq = nc.dram_tensor("wq", [KC, 128, KC, 128], BF16, kind="ExternalInput")
    wk = nc.dram_tensor("wk", [128, KC, KV], BF16, kind="ExternalInput")
    wv = nc.dram_tensor("wv", [128, KC, KV], BF16, kind="ExternalInput")
    wo = nc.dram_tensor("wo", [128, KC, C], BF16, kind="ExternalInput")
    bq = nc.dram_tensor("bq", [128, KC], F32, kind="ExternalInput")
    bk = nc.dram_tensor("bk", [128, 2], F32, kind="ExternalInput")
    ident = nc.dram_tensor("ident", [128, 128], BF16, kind="ExternalInput")
    # mask slots: 0 = kt0@qb0, 1 = kt1@qb0, 2 = kt0@qb>0 (T0), 3 = kt2 (T2)
    maskp = nc.dram_tensor("maskp", [128, 4, NG, 128], BF16, kind="ExternalInput")
    out = nc.dram_tensor("out", [TQ, C], BF16, kind="ExternalOutput")

    with _ChunkedDrainTileContext(nc) as tc:
        with contextlib.ExitStack() as ctx:
            wsb = ctx.enter_context(tc.tile_pool(name="wsb", bufs=1))
            xsb = ctx.enter_context(tc.tile_pool(name="xsb", bufs=1))
            csb = ctx.enter_context(tc.tile_pool(name="csb", bufs=1))
            qkv = ctx.enter_context(tc.tile_pool(name="qkv", bufs=1))
            ynp = ctx.enter_context(tc.tile_pool(name="ynp", bufs=1))
            expp = ctx.enter_context(tc.tile_pool(name="expp", bufs=8))
            rcpp = ctx.enter_context(tc.tile_pool(name="rcpp", bufs=4))
            ytnp = ctx.enter_context(tc.tile_pool(name="ytnp", bufs=6))
            obp = ctx.enter_context(tc.tile_pool(name="obp", bufs=3))
            pj = ctx.enter_context(tc.tile_pool(name="pj", bufs=1, space="PSUM"))
            # av-output y_t [128,4,65] gets its own bank
            ytp_pool = ctx.enter_context(tc.tile_pool(name="ytp", bufs=1, space="PSUM"))
            scp_pool = ctx.enter_context(tc.tile_pool(name="scp", bufs=2, space="PSUM"))

            EARLY = [True]

            def proj_psum():
                # before the attention pipeline starts, projections borrow
                # the scores pool's big tiles (double-buffered); once units
                # are flowing they use the dedicated pj bank so they don't
                # steal the scores rotation.
                if EARLY[0]:
                    t = scp_pool.tile([128, 3, NG, 128], F32, name="scp", tag="sc")
                    return t[:, 0, :, :].rearrange("p g q -> p (g q)")
                return pj.tile([128, 512], F32, name="pp", tag="pj")

            # ---- loads, ordered by consumption deadline ----
            wk_sb = wsb.tile([128, KC, KV], BF16, name="wk_sb", tag="wk")
            nc.sync.dma_start(out=wk_sb[:, 0:4, 0:128], in_=wk[:, 0:4, 0:128])
            xt_sb = xsb.tile([128, KC, TE], BF16, name="xt_sb", tag="xt")
            nc.sync.dma_start(out=xt_sb[:, 0:2, 0:384], in_=xt[:, 0:2, 0:384])
            nc.sync.dma_start(out=xt_sb[:, 2:4, 0:384], in_=xt[:, 2:4, 0:384])
            nc.sync.dma_start(out=wk_sb[:, 4:8, 0:128], in_=wk[:, 4:8, 0:128])
            nc.sync.dma_start(out=xt_sb[:, 4:6, 0:384], in_=xt[:, 4:6, 0:384])
            nc.sync.dma_start(out=xt_sb[:, 6:8, 0:384], in_=xt[:, 6:8, 0:384])
            nc.sync.dma_start(out=wk_sb[:, :, 128:256], in_=wk[:, :, 128:256])
            bk_sb = csb.tile([128, 2], F32)
            nc.sync.dma_start(out=bk_sb[:], in_=bk[:])
            wv_sb = wsb.tile([128, KC, KV], BF16, name="wv_sb", tag="wv")
            nc.sync.dma_start(out=wv_sb[:], in_=wv[:])
            nc.sync.dma_start(out=xt_sb[:, 0:4, 384:TE], in_=xt[:, 0:4, 384:TE])
            nc.sync.dma_start(out=xt_sb[:, 4:8, 384:TE], in_=xt[:, 4:8, 384:TE])
            bq_sb = csb.tile([128, KC], F32)
            nc.sync.dma_start(out=bq_sb[:], in_=bq[:])

            wq_sb = [None] * KC

            def _load_wq(m):
                t = wsb.tile([128, KC, 128], BF16, name=f"wq{m}", tag=f"wq{m}")
                nc.sync.dma_start(out=t[:], in_=wq[m, :, :, :])
                wq_sb[m] = t

            for m in range(2):
                _load_wq(m)
            id_sb = csb.tile([128, 128], BF16)
            nc.sync.dma_start(out=id_sb[:], in_=ident[:])
            mask_sb = csb.tile([128, 4, NG, 128], BF16)
            nc.sync.dma_start(out=mask_sb[:], in_=maskp[:])
            for m in range(2, KC):
                _load_wq(m)
            wo_sb = wsb.tile([128, KC, C], BF16, name="wo_sb", tag="wo")
            nc.sync.dma_start(out=wo_sb[:, :, 0:512], in_=wo[:, :, 0:512])
            nc.sync.dma_start(out=wo_sb[:, :, 512:1024], in_=wo[:, :, 512:1024])

            # ---- kT projection: kT4 [64, NG, TE] bf16 (base-partition 0;
            # bf16 matmuls with operands at base partition 64 crash walrus)
            kT4 = qkv.tile([64, NG, TE], BF16, name="kT4", tag="kT4")

            def k_proj(s2, mt):
                kp = proj_psum()
                for kc in range(KC):
                    nc.tensor.matmul(
                        kp[:, 0:384],
                        wk_sb[:, kc, mt * 128 : (mt + 1) * 128],
                        xt_sb[:, kc, s2 * 384 : (s2 + 1) * 384],
                        start=(kc == 0),
                        stop=(kc == KC - 1),
                    )
                for gh in range(2):
                    g = 2 * mt + gh
                    if gh == 0:
                        nc.vector.tensor_scalar_add(
                            kT4[0:64, g, s2 * 384 : (s2 + 1) * 384],
                            kp[0:64, 0:384],
                            bk_sb[0:64, mt : mt + 1],
                        )
                    else:
                        nc.scalar.activation(
                            kT4[0:64, g, s2 * 384 : (s2 + 1) * 384],
                            kp[64:128, 0:384],
                            mybir.ActivationFunctionType.Identity,
                            bias=bk_sb[64:128, mt : mt + 1],
                        )

            # ---- v projection: token-major [128, NG, 65] with ones column ----
            v_sb = []
            for vt in range(6):
                t = qkv.tile([128, NG, HD + 1], BF16, name=f"v{vt}", tag=f"v{vt}")
                nc.vector.memset(t[:, :, HD : HD + 1], 1.0)
                v_sb.append(t)

            def v_proj(vt):
                vp = proj_psum()
                for kc in range(KC):
                    nc.tensor.matmul(
                        vp[:, 0:KV],
                        xt_sb[:, kc, vt * 128 : (vt + 1) * 128],
                        wv_sb[:, kc, :],
                        start=(kc == 0),
                        stop=(kc == KC - 1),
                    )
                nc.scalar.copy(
                    v_sb[vt][:, :, 0:HD],
                    vp[:, 0:KV].rearrange("p (g d) -> p g d", g=NG),
                )

            yn = ynp.tile([128, KC, TQ], BF16)
            qT_sb = [None] * KC

            def q_proj(m, dve_only=False):
                qp = proj_psum()
                for kc in range(KC):
                    nc.tensor.matmul(
                        qp[:],
                        wq_sb[m][:, kc, :],
                        xt_sb[:, kc, WINDOW:TE],
                        start=(kc == 0),
                        stop=(kc == KC - 1),
                    )
                qT = qkv.tile([64, 2, TQ], BF16, name=f"qT{m}", tag=f"qT{m}")
                nc.vector.tensor_scalar_add(
                    qT[0:64, 0, :], qp[0:64, :], bq_sb[0:64, m : m + 1]
                )
                if dve_only:
                    # early q-projs: ScalarE is exp-busy, DVE is free
                    nc.vector.tensor_scalar_add(
                        qT[0:64, 1, :], qp[64:128, :], bq_sb[64:128, m : m + 1]
                    )
                else:
                    nc.scalar.activation(
                        qT[0:64, 1, :],
                        qp[64:128, :],
                        mybir.ActivationFunctionType.Identity,
                        bias=bq_sb[64:128, m : m + 1],
                    )
                qT_sb[m] = qT

            def attn_scores(g, qb):
                """12 score matmuls + exp + masks; returns masked ex tile."""
                scp = scp_pool.tile([128, 3, NG, 128], F32, name="scp", tag="sc")
                for kt in range(3):
                    ke0 = qb * 128 + kt * 128
                    for mi in range(2):
                        m = 2 * g + mi
                        # 2 heads per matmul: rhs [64, 2, 128] (hi = 2*mi+hh)
                        nc.tensor.matmul(
                            scp[:, kt, 2 * mi : 2 * mi + 2, :],
                            kT4[0:64, g, ke0 : ke0 + 128],
                            qT_sb[m][0:64, :, qb * 128 : qb * 128 + 128],
                            start=True,
                            stop=True,
                        )
                ex = expp.tile([128, 3, NG, 128], BF16, name="ex", tag="ex")
                nc.scalar.activation(ex[:], scp[:], mybir.ActivationFunctionType.Exp)
                return ex

            def attn_masks(g, qb, ex):
                # band masks (multiplicative 0/1): kt0 always, kt1 only at qb0,
                # kt2 always. Middle tile fully valid for qb>0. Emitted AFTER
                # the previous unit's recip/norm so they don't head-of-line
                # block the DVE queue while waiting on exp.
                # edge cores: qb0 kt0/kt1 fully padded; qb1 kt0 also reaches
                # into the padding (slot 0 = T0 on std cores, zeros on edge)
                slot0 = 0 if qb <= 1 else 2
                nc.vector.tensor_tensor(
                    ex[:, 0, :, :], ex[:, 0, :, :], mask_sb[:, slot0, :, :],
                    mybir.AluOpType.mult,
                )
                if qb == 0:
                    nc.gpsimd.tensor_tensor(
                        ex[:, 1, :, :], ex[:, 1, :, :], mask_sb[:, 1, :, :],
                        mybir.AluOpType.mult,
                    )
                nc.vector.tensor_tensor(
                    ex[:, 2, :, :], ex[:, 2, :, :], mask_sb[:, 3, :, :],
                    mybir.AluOpType.mult,
                )

            def attn_av(g, qb, ex):
                """av (transposed) + recip + norm. kt order 1,2,0 so the
                unmasked middle tile starts immediately after exp and the
                slow gpsimd kt0 mask gets maximal slack."""
                comb = ytp_pool.tile([128, 388], F32, name="comb", tag="yt")
                y_t = comb[:, 0:260].rearrange("p (h d) -> p h d", h=NG)
                # hi outer: only one psum accumulation group open per bank
                for hi in range(4):
                    for kt in (1, 0, 2):
                        nc.tensor.matmul(
                            y_t[:, hi, :],
                            ex[:, kt, hi, :],
                            v_sb[qb + kt][:, g, :],
                            start=(kt == 1),
                            stop=(kt == 2),
                        )

                rcp = rcpp.tile([128, NG, 1], F32, name="rcp", tag="rcp")
                with nc.allow_low_precision(reason="softmax denom reciprocal"):
                    nc.vector.reciprocal(rcp[:], y_t[:, :, HD : HD + 1])
                y_n = ytnp.tile([128, NG, HD], BF16, name="y_n", tag="y_n")
                if USE_BCAST:
                    nc.vector.tensor_tensor(
                        y_n[:],
                        y_t[:, :, 0:HD],
                        rcp[:].broadcast_to([128, NG, HD]),
                        mybir.AluOpType.mult,
                    )
                else:
                    for hi in range(4):
                        nc.vector.tensor_scalar_mul(
                            y_n[:, hi, :], y_t[:, hi, 0:HD], rcp[:, hi, :]
                        )
                return comb, y_n

            def attn_tr(g, qb, comb, y_n):
                """transpose back to head-major (bf16 psum region of comb)."""
                if USE_IS_TRANSPOSE:
                    yTp = comb[:, 260:388].bitcast(BF16).rearrange("p (m q) -> p m q", m=2)
                    for mi in range(2):
                        nc.tensor.matmul(
                            yTp[:, mi, :],
                            y_n[:, 2 * mi : 2 * mi + 2, :],
                            id_sb[:],
                            start=True,
                            stop=True,
                            is_transpose=True,
                        )
                else:
                    yTf = pj.tile([128, 512], F32, name="pp", tag="pj")
                    yTp = yTf[:, 0:256].rearrange("p (m q) -> p m q", m=2)
                    for mi in range(2):
                        nc.tensor.matmul(
                            yTp[:, mi, :],
                            y_n[:, 2 * mi : 2 * mi + 2, :],
                            id_sb[:],
                            start=True,
                            stop=True,
                        )
                nc.vector.tensor_copy(
                    yn[:, 2 * g : 2 * g + 2, qb * 128 : qb * 128 + 128], yTp[:]
                )

            ob_cur = [None]

            def out_chunk(tt, n2, borrow=False):
                ob = obp.tile([128, 512], BF16, name="ob", tag="ob")
                if borrow:
                    bt = scp_pool.tile([128, 3, NG, 128], F32, name="scp", tag="sc")
                    op = bt[:, 0, :, :].rearrange("p g q -> p (g q)")
                else:
                    op = pj.tile([128, 512], F32, name="op", tag="pj")
                for m in range(KC):
                    nc.tensor.matmul(
                        op[:],
                        yn[:, m, tt * 128 : (tt + 1) * 128],
                        wo_sb[:, m, n2 * 512 : (n2 + 1) * 512],
                        start=(m == 0),
                        stop=(m == KC - 1),
                    )
                nc.scalar.copy(ob[:], op[:])
                nc.sync.dma_start(
                    out=out[tt * 128 : (tt + 1) * 128, n2 * 512 : (n2 + 1) * 512],
                    in_=ob[:],
                )

            # ---- schedule ----
            # unit (g, qb) pipeline with lag-1 av and lag-2 transpose;
            # proj chunks as filler at the head of each unit (they absorb the
            # exp->scores psum-reuse wait).
            # mid-fillers sit between av(prev) and tr(prev) to absorb the
            # DVE recip/norm latency with PE work.
            mid = {
                (1, 1): [lambda: out_chunk(0, 0)],
                (2, 1): [lambda: out_chunk(0, 1)],
                (1, 2): [lambda: out_chunk(1, 0)],
                (2, 2): [lambda: out_chunk(1, 1)],
                (1, 3): [lambda: out_chunk(2, 0)],
                (2, 3): [lambda: out_chunk(2, 1)],
            }
            pend = None  # (g, qb, ex) awaiting av+tr

            # unit pipeline: scores(u) | av(prev)+recip/norm | masks(u) |
            # filler | transpose(prev). qb0 interleaves the q/v projections;
            # qb>=1 uses out-proj chunks as mid fillers.
            mid_f = {
                (1, 1): [lambda: out_chunk(0, 0)],
                (2, 1): [lambda: out_chunk(0, 1)],
                (1, 2): [lambda: out_chunk(1, 0)],
                (2, 2): [lambda: out_chunk(1, 1)],
                (1, 3): [lambda: out_chunk(2, 0)],
                (2, 3): [lambda: out_chunk(2, 1)],
            }
            PHASE = int(os.environ.get("KV2_PHASE", "5"))
            k_proj(0, 0)
            k_proj(0, 1)
            v_proj(0)
            v_proj(1)
            v_proj(2)
            k_proj(1, 0)
            k_proj(1, 1)
            q_proj(0)
            q_proj(1)
            if PHASE == 1:
                for m in range(2, KC):
                    q_proj(m)
                nc.sync.dma_start(out=out[0:128, 0:512], in_=qT_sb[0][:])
            for g in (range(NG) if PHASE >= 2 else []):
                ex = attn_scores(g, 0)
                if g < 3:
                    q_proj(2 * g + 2)
                if PHASE >= 3 and pend is not None:
                    comb, y_n = attn_av(*pend)
                    attn_masks(g, 0, ex)
                    if g < 3:
                        q_proj(2 * g + 3)
                    v_proj(g + 2)
                    attn_tr(pend[0], pend[1], comb, y_n)
                else:
                    attn_masks(g, 0, ex)
                    if g == 0:
                        q_proj(3)
                    elif g < 3:
                        q_proj(2 * g + 3)
                        v_proj(g + 2)
                pend = (g, 0, ex)
            if PHASE == 2:
                nc.sync.dma_start(out=out[0:128, 0:512], in_=pend[2][:, 0, :, :].rearrange("p a b -> p (a b)"))
            EARLY[0] = False
            if PHASE == 3:
                comb, y_n = attn_av(*pend)
                attn_tr(pend[0], pend[1], comb, y_n)
                nc.sync.dma_start(out=out[0:128, 0:512], in_=yn[:, 0, :])
            for qb in (range(1, 4) if PHASE >= 4 else []):
                for g in range(NG):
                    u = (g, qb)
                    ex = attn_scores(*u)
                    comb, y_n = attn_av(*pend)
                    attn_masks(u[0], u[1], ex)
                    if PHASE >= 5:
                        for f in mid_f.get(u, []):
                            f()
                    attn_tr(pend[0], pend[1], comb, y_n)
                    pend = (u[0], u[1], ex)
            if PHASE >= 4:
                comb, y_n = attn_av(*pend)
            if PHASE >= 5:
                # partial out(3,0) over groups 0-2 (their qb3 tails are done)
                # accumulates in a borrowed scores bank while the last tail's
                # DVE/ScalarE chain drains; heads 6-7 finish after the tr.
                bt30 = scp_pool.tile([128, 3, NG, 128], F32, name="scp", tag="sc")
                op30 = bt30[:, 0, :, :].rearrange("p g q -> p (g q)")
                for m in range(6):
                    nc.tensor.matmul(
                        op30[:],
                        yn[:, m, 384:512],
                        wo_sb[:, m, 0:512],
                        start=(m == 0),
                        stop=False,
                    )
            if PHASE >= 4:
                attn_tr(pend[0], pend[1], comb, y_n)
            if PHASE == 4:
                nc.sync.dma_start(out=out[0:128, 0:512], in_=yn[:, 0, :])
            if PHASE >= 5:
                for m in (6, 7):
                    nc.tensor.matmul(
                        op30[:],
                        yn[:, m, 384:512],
                        wo_sb[:, m, 0:512],
                        start=False,
                        stop=(m == 7),
                    )
                ob30 = obp.tile([128, 512], BF16, name="ob", tag="ob")
                nc.scalar.copy(ob30[:], op30[:])
                nc.sync.dma_start(out=out[384:512, 0:512], in_=ob30[:])
                out_chunk(3, 1, borrow=True)

    _split_multi_waits(nc)
    return nc


_NC = None


def _get_nc():
    global _NC
    if _NC is None:
        _NC = _build_program()
    return _NC


def _to_bf16(a):
    import ml_dtypes

    return np.asarray(a, np.float32).astype(ml_dtypes.bfloat16)


def _to_fp8(a):
    from concourse import mybir as _mb

    return np.asarray(a, np.float32).astype(_mb.dt.np(_mb.dt.float8e4))


def _host_prep(x, Wq, bq, Wk, bk, Wv, bv, Wo, bo):
    x = np.asarray(x, np.float32)
    Wq = np.asarray(Wq, np.float32)
    bq = np.asarray(bq, np.float32)
    Wk = np.asarray(Wk, np.float32)
    bk = np.asarray(bk, np.float32)
    Wv = np.asarray(Wv, np.float32)
    bv = np.asarray(bv, np.float32)
    Wo = np.asarray(Wo, np.float32)
    bo = np.asarray(bo, np.float32)

    scale = np.float32(1.0 / np.sqrt(HD))
    # wq[m][p, kc, n] = Wq[kc*128+p, m*128+n] * scale
    wq_t = _to_bf16(
        np.ascontiguousarray((Wq * scale).reshape(KC, 128, KC, 128).transpose(2, 1, 0, 3))
    )
    wk_t = _to_bf16(np.ascontiguousarray(Wk.reshape(KC, 128, KV).transpose(1, 0, 2)))
    wv_t = _to_bf16(np.ascontiguousarray(Wv.reshape(KC, 128, KV).transpose(1, 0, 2)))
    wo_t = _to_bf16(np.ascontiguousarray(Wo.reshape(KC, 128, C).transpose(1, 0, 2)))
    # bq_sb[p, m] = bq[m*128+p] * scale
    bq_t = np.ascontiguousarray((bq * scale).reshape(KC, 128).T)
    # bk_sb[gh*64+d, mt] = bk[(2*mt+gh)*64+d]
    bk_t = np.ascontiguousarray(bk.reshape(2, 2, 64).transpose(1, 2, 0).reshape(128, 2))
    ident = _to_bf16(np.eye(128, dtype=np.float32))

    # masks [128 kj, slot, g(replicated), 128 qi]
    kj = np.arange(128)[:, None]
    qi = np.arange(128)[None, :]
    t0 = (kj >= qi).astype(np.float32)  # kt0 (and qb0-interior)
    t2 = (kj <= qi).astype(np.float32)  # kt2
    ones = np.ones((128, 128), np.float32)
    zeros = np.zeros((128, 128), np.float32)

    def mk_mask(edge):
        # slots: 0 = kt0@qb0, 1 = kt1@qb0, 2 = kt0@qb>0, 3 = kt2
        s0 = zeros if edge else t0
        s1 = zeros if edge else ones
        m = np.stack([s0, s1, t0, t2])  # [4, 128, 128]
        return _to_bf16(np.broadcast_to(m[None, :, :, :], (NG, 4, 128, 128)).transpose(2, 1, 0, 3).copy())

    mask_std = mk_mask(False)
    mask_edge = mk_mask(True)

    in_maps = []
    for core in range(NCORES):
        b, c = core // 4, core % 4
        t0c = c * TQ - WINDOW
        xe = np.zeros((TE, C), np.float32)
        lo = max(t0c, 0)
        xe[lo - t0c : TE, :] = x[b, lo : t0c + TE, :]
        xt_t = _to_bf16(np.ascontiguousarray(xe.T.reshape(KC, 128, TE).transpose(1, 0, 2)))
        in_maps.append(
            {
                "xt": xt_t,
                "wq": wq_t,
                "wk": wk_t,
                "wv": wv_t,
                "wo": wo_t,
                "bq": bq_t,
                "bk": bk_t,
                "ident": ident,
                "maskp": mask_edge if c == 0 else mask_std,
            }
        )

    bv_rep = np.concatenate([bv[(h // NG) * HD : (h // NG + 1) * HD] for h in range(NH)])
    corr = bv_rep.astype(np.float64) @ Wo.astype(np.float64) + bo.astype(np.float64)
    return in_maps, corr.astype(np.float32)


LAST_RESULTS = None


def kernel(x, Wq, bq, Wk, bk, Wv, bv, Wo, bo):
    global LAST_RESULTS
    in_maps, corr = _host_prep(x, Wq, bq, Wk, bk, Wv, bv, Wo, bo)
    nc = _get_nc()
    res = run_bass_kernel_spmd(nc, in_maps, core_ids=list(range(NCORES)))
    LAST_RESULTS = res
    out = np.empty((B, T, C), np.float32)
    for core in range(NCORES):
        b, c = core // 4, core % 4
        out[b, c * TQ : (c + 1) * TQ, :] = res.results[core]["out"].astype(np.float32)
    out += corr[None, None, :]
    return out
